# revision 53
# baseline (speedup 1.0000x reference)
"""Trainium2 Bass kernel for the quirky MultiHeadAttention module.

Reference computation (B=4, S=1024, H=768, NH=12, HS=64):
    Q = (x@Wq+bq)  split into heads     [B,12,S,64]
    K = (x@Wk+bk)  split into heads     [B,12,S,64]
    V = x@Wv+bv    NOT split            [B,S,768]
    A = softmax(QK^T/8 + mask)          [B,12,S,S]
    out = (A @ V) reshaped [B, S*12, H] @ Wo + bo    -> [4, 12288, 768]

Decomposition (device = the attention core, 83% of the FLOPs):
  * (A @ V) @ Wo = A @ (x @ (Wv@Wo)) (+ cvec = bv@Wo + bo, folded into
    every VW row -- softmax rows sum to one, so A @ (VW + cvec_row)
    = A@VW + sigma*cvec).
  * Masked keys produce exp(-1e9+s) == 0 exactly, so they are dropped on
    the host and the key axis is compacted per batch.
  * The projections Q = x@Wq+bq, K = x@Wk+bk, VW = x@(Wv@Wo)+cvec are
    HOST-side f32 GEMMs (~10 GFLOP, an extension of the baseline's
    host-side Wv@Wo fold); the device receives packed f16 Q^T/K^T/VW
    blobs (1.9 MB/core vs 4.7 MB of raw x+weights) and runs only
    scores -> exp -> PV, emitting UNNORMALIZED numerators + a sigma
    column in f16; the host performs the final division (which makes
    partial-sum sharding legal).
  * The device covers at most bkt=4 key tiles (512 compacted keys) per
    batch; the few overflow keys of a denser batch (here 17) contribute
    their partial numerator/sigma on the HOST (exact f32).

Sharding: 8 cores = 4 batches x 2 head-groups (6 heads each), pure SPMD.

Device layouts (all pre-transposed so no on-device transposes):
    QT/KT: [feat, tok] with head pairs packed 64+64 in partitions; the
        64-row score matmuls run 2-head-concurrent via PE row groups.
    S^T = KT.T @ QT -> [k, q]  (k on partitions => mask is a per-partition
        bias folded into the Exp activation)
    U = exp(S^T) [k, q] f16 -> exactly the layout the PV matmul needs
    out = U.T @ [VW | 1] -> [q, 770] with cols 768:770 = sigma, f16.
Each PV group uses two single-bank PSUM tiles whose drains (2:1
vector:scalar) interleave with the matmul chains; output DMAs are one
per (chunk, head-half) on the sync/gpsimd queues, except the last
chunk which DMAs per mq on the low-latency HW queues for a short tail.
A burst of junk matmuls pre-warms the PE clock gate (HAM) to 2.4 GHz
while the first QT/KT blobs stream in.
"""

import math

import numpy as np

B, S, H, NH, HS = 4, 1024, 768, 12, 64
GW = 384          # head-group width = 6 heads * 64
NCORES = 8
BKT_CAP = 4       # device covers at most 4 key tiles; rest goes to host

_PROGRAM_CACHE = {}


def _build_program(bkt):
    """bkt: number of 128-wide key tiles per core (1..4)."""
    import concourse.mybir as mybir
    import concourse.tile as tile
    from concourse import bacc
    from concourse.bass import ds, ts

    f32 = mybir.dt.float32
    f16 = mybir.dt.float16
    AF = mybir.ActivationFunctionType

    KMAX = 128 * bkt

    nc = bacc.Bacc(None, target_bir_lowering=False, debug=False)

    qt_d = nc.dram_tensor("qt", (128, 3, 1024), f16, kind="ExternalInput")
    kt_d = nc.dram_tensor("kt", (128, 3, KMAX), f16, kind="ExternalInput")
    vw_d = nc.dram_tensor("vw", (128, bkt, 770), f16, kind="ExternalInput")
    sv_d = nc.dram_tensor("sv", (128, bkt), f32, kind="ExternalInput")
    # layout [j, hh, qc, p, mq, f]: DMA src iterates (partition, mq-block,
    # f), so those must be the three minor dims of the destination
    out_d = nc.dram_tensor("out", (3, 2, 2, 128, 4, 770), f16,
                           kind="ExternalOutput")

    with tile.TileContext(nc) as tc:
        with (
            tc.tile_pool(name="persist", bufs=1) as pp,
            tc.tile_pool(name="ut", bufs=4 * max(bkt, 2)) as utp,
            tc.tile_pool(name="osb", bufs=6) as op_,
            tc.tile_pool(name="psS", bufs=4, space="PSUM") as psSp,
            tc.tile_pool(name="psO", bufs=4, space="PSUM") as psOp,
        ):
            # ---- stream inputs (order within each queue = priority) ----
            sv = pp.tile([128, bkt], f32, name="sv", tag="sv")
            nc.sync.dma_start(sv[:], sv_d[:])
            mk_t = [sv[:, k:k + 1] for k in range(bkt)]

            QT = [pp.tile([128, 1024], f16, name=f"QT{j}", tag=f"QT{j}")
                  for j in range(3)]
            KT = [pp.tile([128, KMAX], f16, name=f"KT{j}", tag=f"KT{j}")
                  for j in range(3)]
            VWB = pp.tile([128, bkt, 770], f16, name="VWB", tag="VWB")
            VW = [VWB[:, m, :] for m in range(bkt)]

            # Small early blobs ride the low-latency HW queues (SWDGE
            # pays ~3us fixed per DMA); the big VW blob rides SWDGE.
            # chunk 0 = (j0, qc0): kt0 + first qt0 half first.
            nc.sync.dma_start(QT[0][:, 0:512], qt_d[:, 0, 0:512])
            nc.scalar.dma_start(KT[0][:], kt_d[:, 0, :])
            h = (bkt + 1) // 2
            nc.gpsimd.dma_start(VWB[:, 0:h, :], vw_d[:, 0:h, :])
            nc.gpsimd.dma_start(VWB[:, h:bkt, :], vw_d[:, h:bkt, :])
            nc.sync.dma_start(QT[0][:, 512:1024], qt_d[:, 0, 512:1024])
            nc.scalar.dma_start(KT[1][:], kt_d[:, 1, :])
            nc.sync.dma_start(QT[1][:], qt_d[:, 1, :])
            nc.scalar.dma_start(KT[2][:], kt_d[:, 2, :])
            nc.sync.dma_start(QT[2][:], qt_d[:, 2, :])

            def emit_scores(ch, sp):
                """Score MMs kt-major (2-head row-group concurrency),
                then exps hh-major so the first PV group unblocks
                after only nkt exps. Fills ch['ut']."""
                nkt = len(ch["kt_sb"])
                qt_sb, qch, masks = ch["qt"], ch["qch"], ch["masks"]
                psS = [[None] * nkt for _ in range(2)]
                for i in range(nkt):
                    ktile, csel, _vw = ch["kt_sb"][i]
                    for hh in range(2):
                        p0 = hh * 64
                        ps = sp.tile([128, 512], f32, name="psS",
                                     tag="psS")
                        nc.tensor.matmul(
                            ps[:], ktile[p0:p0 + 64, csel],
                            qt_sb[p0:p0 + 64, qch])
                        psS[hh][i] = ps
                ut = [[None] * nkt for _ in range(2)]
                # exps in MM allocation order (kt-major): score MM i+4
                # then waits exp i (early), never a late exp -- the psS
                # ring recycles without stalling the hoisted score MMs.
                # (The PV side can afford it: exps are hoisted half a
                # chunk early, so waiting 2*nkt exps instead of nkt
                # costs nothing.)
                order = [(hh, i) for i in range(nkt)
                         for hh in range(2)]
                for hh, i in order:
                    u = utp.tile([128, 512], f16, name="ut", tag="ut")
                    nc.scalar.activation(
                        u[:], psS[hh][i][:], AF.Exp, bias=masks[i])
                    ut[hh][i] = u
                ch["ut"] = ut

            base_rings = (nc.sync, nc.gpsimd)
            chunks = []
            for ci, (j, qc) in enumerate(
                    (j, qc) for j in range(3) for qc in range(2)):
                def odst1(mq, h2, hh, j=j, qc=qc):
                    return out_d[j, hh, qc, :, ds(mq, 1), ds(h2 * 385, 385)]
                def odstf(hh, j=j, qc=qc):
                    return out_d[j, hh, qc, :, :, :]
                chunks.append(dict(
                    qch=ds(qc * 512, 512),
                    kt_sb=[(KT[j], ts(kt, 128), VW[kt])
                           for kt in range(bkt)],
                    qt=QT[j], masks=mk_t, odst1=odst1, odstf=odstf,
                    last=False, rings=base_rings))
            # last chunk: finer DMA granularity on the two HW queues
            # (low completion latency; SWDGE would add ~3us to the tail)
            chunks[-1]["rings"] = (nc.scalar, nc.sync)
            chunks[-1]["last"] = True

            # PE warm-up: junk matmuls keep the tensor engine busy
            # through the HAM SHORT window (~3.4us) so real work runs at
            # 2.4 GHz from the start (count tuned to end ~when the first
            # kt/qt blobs land).
            wsrc = pp.tile([128, 384], f16, name="wsrc", tag="wsrc")
            nc.vector.memset(wsrc[:], 0.0)
            for _ in range(42):
                psw = psSp.tile([64, 128], f32, name="warm", tag="psS")
                nc.tensor.matmul(psw[:], wsrc[:, 0:64], wsrc[:, 0:128])

            ring_i = [0]

            def out_dma(dst, src, rings):
                rings[ring_i[0] % len(rings)].dma_start(dst, src)
                ring_i[0] += 1

            drain_i = [0]
            zb = pp.tile([128, 1], f32, name="zb", tag="zb")
            nc.gpsimd.memset(zb[:], 0.0)

            def drain(dst, src):
                # PSUM->SBUF drains 2:1 vector:scalar -- keeps the vector
                # queue short so psO banks recycle without stalling PV
                if drain_i[0] % 3 < 2:
                    nc.vector.tensor_copy(dst, src)
                else:
                    nc.scalar.activation(dst, src, AF.Identity, bias=zb[:])
                drain_i[0] += 1

            def emit_pv(ch, nxt):
                """PV groups hh-major; each group accumulates into one
                2-bank PSUM tile (pa bank 0, pb bank-aligned at col
                512), drained by ONE strided cast; output DMAs go ONE
                per (hh) (per mq on the last chunk for a short tail).
                The NEXT chunk's scores+exps are hoisted in between the
                hh halves so its exps complete during this chunk's PV
                (a chunk-boundary exp wait re-throttles the PE clock)."""
                nkt = len(ch["kt_sb"])
                ut, rings = ch["ut"], ch["rings"]
                for hh in range(2):
                    oh = op_.tile([128, 4, 770], f16, name="ob", tag="ob")
                    for mq in range(4):
                        # two single-bank psum tiles per group: pb drains
                        # DURING the pa chain, so banks recycle a whole
                        # chain earlier than a fused 2-bank drain would
                        pob = psOp.tile([128, 512], f32, name="psOb",
                                        tag="psO")
                        poa = psOp.tile([128, 512], f32, name="psOa",
                                        tag="psO")
                        for i in range(nkt):
                            nc.tensor.matmul(
                                pob[:, 0:385], ut[hh][i][:, ts(mq, 128)],
                                ch["kt_sb"][i][2][:, 385:770],
                                start=(i == 0), stop=(i == nkt - 1))
                        drain(oh[:, mq, 385:770], pob[:, 0:385])
                        for i in range(nkt):
                            nc.tensor.matmul(
                                poa[:, 0:385], ut[hh][i][:, ts(mq, 128)],
                                ch["kt_sb"][i][2][:, 0:385],
                                start=(i == 0), stop=(i == nkt - 1))
                        drain(oh[:, mq, 0:385], poa[:, 0:385])
                        if ch["last"]:
                            out_dma(ch["odst1"](mq, 1, hh),
                                    oh[:, ds(mq, 1), ds(385, 385)], rings)
                            out_dma(ch["odst1"](mq, 0, hh),
                                    oh[:, ds(mq, 1), ds(0, 385)], rings)
                    if not ch["last"]:
                        out_dma(ch["odstf"](hh), oh[:], rings)
                    if hh == 0 and nxt is not None:
                        emit_scores(nxt, psSp)

            emit_scores(chunks[0], psSp)
            for ci, ch in enumerate(chunks):
                emit_pv(ch, chunks[ci + 1] if ci + 1 < len(chunks)
                        else None)
    nc.compile()
    return nc


def get_program(bkt):
    if bkt not in _PROGRAM_CACHE:
        _PROGRAM_CACHE[bkt] = _build_program(bkt)
    return _PROGRAM_CACHE[bkt]


def prep(x, mask, Wq, bq, Wk, bk, Wv, bv, Wo, bo):
    """Host-side projections + sharding/compaction. Returns (bkt,
    in_maps, perms, host_ctx)."""
    f16 = np.float16
    x = np.asarray(x, np.float32)
    mask = np.asarray(mask)
    Wq = np.asarray(Wq, np.float32)
    Wk = np.asarray(Wk, np.float32)
    Wv = np.asarray(Wv, np.float32)
    Wo = np.asarray(Wo, np.float32)
    bq = np.asarray(bq, np.float32)
    bk = np.asarray(bk, np.float32)
    bv = np.asarray(bv, np.float32)
    bo = np.asarray(bo, np.float32)

    mrow = [mask[b, 0, 0] != 0 for b in range(B)]
    perms = [np.argsort(~mrow[b], kind="stable") for b in range(B)]
    nkeep = [int(mrow[b].sum()) for b in range(B)]
    tb = [min(8, max(1, math.ceil(n / 128))) for n in nkeep]
    bkt = min(max(tb), BKT_CAP)
    KMAX = 128 * bkt

    cvec = bv @ Wo + bo
    WvWo = Wv @ Wo

    def pack3(a):
        """[N, 384] -> [128, 3, N] (pair j at [:, j, :])."""
        return np.ascontiguousarray(
            a.T.reshape(3, 128, -1).transpose(1, 0, 2))

    in_maps = [dict() for _ in range(NCORES)]
    for b in range(B):
        xp = x[b][perms[b]]                       # [S, 768] permuted
        Qp = ((xp @ Wq + bq) * 0.125).astype(f16)
        Kp = (xp[:KMAX] @ Wk + bk).astype(f16)
        vwf = (xp[:KMAX] @ WvWo + cvec).astype(f16)   # [KMAX, 768]
        vwb = np.ones((128, bkt, 770), f16)
        vwb[:, :, 0:768] = vwf.reshape(bkt, 128, 768).transpose(1, 0, 2)
        mk = np.full(KMAX, -1e9, np.float32)
        mk[:min(nkeep[b], KMAX)] = 0.0
        mkb = np.ascontiguousarray(mk.reshape(bkt, 128).T)
        for g in range(2):
            c = 2 * b + g
            cs = slice(g * GW, (g + 1) * GW)
            in_maps[c]["qt"] = pack3(Qp[:, cs])
            in_maps[c]["kt"] = pack3(Kp[:, cs])
            in_maps[c]["vw"] = vwb
            in_maps[c]["sv"] = mkb

    # host-side overflow keys (compacted indices beyond KMAX)
    ov = []
    for b in range(B):
        if nkeep[b] > KMAX:
            ov.append((b, x[b][perms[b][KMAX:nkeep[b]]]))
    host_ctx = dict(x=x, Wq=Wq, bq=bq, Wk=Wk, bk=bk, WvWo=WvWo,
                    cvec=cvec, ov=ov)
    return bkt, in_maps, perms, host_ctx


def gather_output(results, perms, host_ctx):
    num = np.zeros((B, NH, S, 768), np.float32)
    sig = np.zeros((B, NH, S, 1), np.float32)
    def unshuffle(o):
        # [.., hh, qc, p, mq, f] -> [.., hh, q, f] with q = qc*512+mq*128+p
        o = o.transpose(0, 1, 2, 4, 3, 5)        # [j, hh, qc, mq, p, f]
        return o.reshape(o.shape[0], 2, 1024, 770)

    for c in range(NCORES):
        b, g = c // 2, c % 2
        o = unshuffle(np.asarray(results[c]["out"], np.float32))
        for j in range(3):
            for hh in range(2):
                h = g * 6 + j * 2 + hh
                num[b, h] += o[j, hh, :, :768]
                sig[b, h, :, 0] += o[j, hh, :, 768]

    # host partial sums for overflow keys (exact f32)
    if host_ctx["ov"]:
        x, Wq, bq = host_ctx["x"], host_ctx["Wq"], host_ctx["bq"]
        Wk, bk = host_ctx["Wk"], host_ctx["bk"]
        WvWo, cvec = host_ctx["WvWo"], host_ctx["cvec"]
        for b, xe in host_ctx["ov"]:
            # device q-axis order == permuted token order
            Q = x[b][perms[b]] @ Wq + bq       # [S, 768]
            Ke = xe @ Wk + bk                  # [ne, 768]
            Ve = xe @ WvWo                     # [ne, 768] (+cvec via sig)
            Qh = Q.reshape(S, NH, HS)
            Kh = Ke.reshape(-1, NH, HS)
            se = np.einsum('qhd,khd->hqk', Qh, Kh) / np.sqrt(
                np.float32(HS))
            ue = np.exp(se)                    # [NH, S, ne]
            num[b] += ue @ Ve + ue.sum(-1, keepdims=True) * cvec
            sig[b, :, :, 0] += ue.sum(-1)

    res = num / sig                                    # [B,NH,S,H]
    out = np.empty((B, S * NH, H), np.float32)
    ov = out.reshape(B, S, NH, H)
    for b in range(B):
        ov[b, perms[b]] = res[b].transpose(1, 0, 2)
    return out


def kernel(**inputs):
    from concourse.bass_utils import run_bass_kernel_spmd

    bkt, in_maps, perms, host_ctx = prep(**inputs)
    nc = get_program(bkt)
    res = run_bass_kernel_spmd(nc, in_maps, core_ids=list(range(NCORES)))
    return gather_output(res.results, perms, host_ctx)


if __name__ == "__main__":
    rng = np.random.default_rng(0)
    demo = {
        "x": rng.standard_normal((B, S, H), dtype=np.float32),
        "mask": rng.integers(0, 2, (B, 1, 1, S)).astype(np.int32),
        "Wq": rng.standard_normal((H, H), dtype=np.float32) / np.sqrt(H),
        "bq": np.zeros(H, np.float32),
        "Wk": rng.standard_normal((H, H), dtype=np.float32) / np.sqrt(H),
        "bk": np.zeros(H, np.float32),
        "Wv": rng.standard_normal((H, H), dtype=np.float32) / np.sqrt(H),
        "bv": np.zeros(H, np.float32),
        "Wo": rng.standard_normal((H, H), dtype=np.float32) / np.sqrt(H),
        "bo": np.zeros(H, np.float32),
    }
    out = kernel(**demo)
    print("kernel ran, output shape", out.shape)
